# revision 1
# baseline (speedup 1.0000x reference)
"""Trainium2 Bass kernel for a single DeBERTa-style attention head.

Problem shapes (hardcoded):
  B=8, S=2048, E=768(n_embed), H=64(head)
  q = I @ Wq + bq ; k = x @ Wk + bk ; v = x @ Wv + bv
  w = (q @ k^T) / sqrt(E) ; w = where(mask==0, -1e9, w)
  scores = softmax(w, axis=-1) ; out = scores @ v

Sharding: data-parallel over batch B across the 8 NeuronCores (one batch
element per core, identical SPMD program). Host-side (inside kernel()) the
per-core slices are laid out transposed (I^T, x^T, mask^T) so the device
never has to transpose bulk data: PE transposes cost ~300ns per 128x128
block and suppress the PE clock-gate warmup, which dominated the v1 profile.

Per-core dataflow (bf16 operands, fp32 PSUM accumulation):
  1. Cast-DMA (SWDGE fp32->bf16) I^T, x^T into SBUF with embed on partitions.
  2. qT,kT [64,2048] = Wq/Wk-chunk (stationary) x I^T/x^T (streaming) + rank-1
     bias matmul; v per k-chunk + bias + a ones column (v_aug) so the softmax
     denominator falls out of the second matmul's extra output column.
  3. k-chunk-major attention: w^T-chunk [128k, q] = kT-chunk^T @ qT;
     e = exp(w^T * 1/sqrt(E)) on ACT straight from PSUM (no row max needed:
     |w/sqrt(E)| is O(1) so exp cannot overflow, and softmax is
     shift-invariant); s^T = e * mask^T (mask int32 DMA-cast to bf16;
     multiplicative masking matches the reference's -1e9 additive mask, which
     underflows to exactly 0 after softmax); ctx[q-chunk, 0:65] accumulates
     s^T-chunk^T @ v_aug over all 16 k-chunks in PSUM.
  4. out = ctx[:,0:64] * (1/ctx[:,64]).
"""

import math
from contextlib import ExitStack

import numpy as np

import concourse.bass as bass
import concourse.tile as tile
import concourse.mybir as mybir
from concourse import bacc
from concourse.bass_utils import run_bass_kernel_spmd

B, S, E, H = 8, 2048, 768, 64
N_CORES = 8
SC = S // 128   # 16 seq chunks
EC = E // 128   # 6 embed chunks
SCALE = 1.0 / math.sqrt(E)

F32 = mybir.dt.float32
BF16 = mybir.dt.bfloat16
I32 = mybir.dt.int32
AF = mybir.ActivationFunctionType
ALU = mybir.AluOpType

_cache = {}


def _build_program():
    nc = bacc.Bacc("TRN2", target_bir_lowering=False, debug=False)

    # Host feeds these already transposed: IT/XT are [E, S], maskT is [S, S]
    # with [k, q] indexing, packed to uint8 (values are 0/1, so the cast is
    # lossless) — 4MB of HBM reads instead of 16MB, expanded to bf16 by the
    # cast-DMA on the way into SBUF.
    dIT = nc.dram_tensor("IT", [E, S], F32, kind="ExternalInput")
    dXT = nc.dram_tensor("XT", [E, S], F32, kind="ExternalInput")
    dmT = nc.dram_tensor("maskT", [S, S], mybir.dt.uint8, kind="ExternalInput")
    # weights host-packed into one contiguous [E, 3H] bf16 tensor and biases
    # into one [H, 2H] f32 tensor (bq/bk broadcast along rows so each
    # partition moves one contiguous run; only col 0 / col H is read)
    dW = nc.dram_tensor("Wpack", [E, 3 * H], BF16, kind="ExternalInput")
    dB = nc.dram_tensor("bpack", [H, 2 * H], F32, kind="ExternalInput")
    dbv = nc.dram_tensor("bv", [1, H], BF16, kind="ExternalInput")
    dout = nc.dram_tensor("out", [S, H], F32, kind="ExternalOutput")

    with tile.TileContext(nc) as tc, ExitStack() as ctx:
        singles = ctx.enter_context(tc.tile_pool(name="singles", bufs=1))

        # SWDGE FIFO order = consumption order: all of I (q path), then x in
        # halves (k/v path), then the mask chunks. The kernel end is paced by
        # the last mask byte, with compute draining right behind it.
        IT = singles.tile([128, EC, S], BF16, tag="IT")
        XT = singles.tile([128, EC, S], BF16, tag="XT")
        nc.gpsimd.dma_start(
            out=IT, in_=dIT.ap().rearrange("(ec p) s -> p ec s", p=128)
        )
        for lo, hi in ((0, S // 2), (S // 2, S)):
            nc.gpsimd.dma_start(
                out=XT[:, :, lo:hi],
                in_=dXT.ap()[:, lo:hi].rearrange("(ec p) s -> p ec s", p=128),
            )

        ones_row = singles.tile([1, S], BF16, tag="ones")
        nc.vector.memset(ones_row, 1.0)

        w_all = singles.tile([128, EC, 3 * H], BF16, tag="Wpack")
        nc.sync.dma_start(
            out=w_all, in_=dW.ap().rearrange("(ec p) h -> p ec h", p=128)
        )
        w_sb = {
            "Wq": w_all[:, :, 0:H],
            "Wk": w_all[:, :, H:2 * H],
            "Wv": w_all[:, :, 2 * H:3 * H],
        }
        b_all = singles.tile([H, 2 * H], F32, tag="bpack")
        nc.sync.dma_start(out=b_all, in_=dB.ap())
        bv_t = singles.tile([1, H], BF16, tag="bv")
        nc.sync.dma_start(out=bv_t, in_=dbv.ap())
        b_sb = {"bq": b_all[:, 0:1], "bk": b_all[:, H:H + 1], "bv": bv_t}

        qT = singles.tile([64, S], BF16, tag="qT")
        kT = singles.tile([64, S], BF16, tag="kT")
        vA = singles.tile([128, SC, 66], BF16, tag="vA")

        # whole mask^T resident (bf16, 64KB/partition), filled by upfront
        # cast-DMAs from the uint8 source — no consumer-slot gating on the
        # stream, which otherwise serializes DMA latency into the TT chain
        maskT_all = singles.tile([128, SC, S], BF16, tag="maskT")
        for ki in range(0, SC, 2):
            nc.gpsimd.dma_start(
                out=maskT_all[:, ki:ki + 2, :],
                in_=dmT.ap()[ki * 128:(ki + 2) * 128, :].rearrange(
                    "(t p) q -> p t q", p=128
                ),
            )

        psw = ctx.enter_context(tc.tile_pool(name="psw", bufs=2, space="PSUM"))
        sp = ctx.enter_context(tc.tile_pool(name="sp", bufs=9))
        eep = ctx.enter_context(tc.tile_pool(name="eep", bufs=3))
        outp = ctx.enter_context(tc.tile_pool(name="outp", bufs=1))

        def emit_score(ki):
            """w^T-chunk -> exp -> mask multiply; returns the sT tile.

            Both w halves are emitted before both exps (and both exps before
            both multiplies) so each engine sees its two ops back-to-back —
            the inter-op pipeline drain overlaps the other half's work on the
            neighbouring engines instead of serializing the chain."""
            maskT_sb = maskT_all[:, ki, :]
            sT_sb = sp.tile([128, S], BF16, tag="sT")
            wps = []
            for hh in range(2):
                wp = psw.tile([128, 1024], F32, tag="w")
                for nb in range(2):
                    nc.tensor.matmul(
                        wp[:, nb * 512:(nb + 1) * 512],
                        lhsT=kT[:, ki * 128:(ki + 1) * 128],
                        rhs=qT[:, (hh * 2 + nb) * 512:(hh * 2 + nb + 1) * 512],
                        start=True,
                        stop=True,
                    )
                wps.append(wp)
            e_sbs = []
            for hh in range(2):
                e_sb = eep.tile([128, 1024], BF16, tag="e")
                nc.scalar.activation(e_sb, wps[hh], AF.Exp, scale=SCALE)
                e_sbs.append(e_sb)
            for hh in range(2):
                nc.vector.tensor_tensor(
                    sT_sb[:, hh * 1024:(hh + 1) * 1024],
                    e_sbs[hh],
                    maskT_sb[:, hh * 1024:(hh + 1) * 1024],
                    ALU.mult,
                )
            return sT_sb

        def emit_qk_chunk(wname, bname, dstT, srcT, nb):
            ps = ps2.tile([64, 512], F32, tag="pqk")
            for ei in range(EC):
                nc.tensor.matmul(
                    ps,
                    lhsT=w_sb[wname][:, ei, :],
                    rhs=srcT[:, ei, nb * 512:(nb + 1) * 512],
                    start=(ei == 0),
                    stop=(ei == EC - 1),
                )
            # bias folded into the PSUM->SBUF copy on DVE (per-partition add)
            nc.vector.tensor_scalar(
                dstT[:, nb * 512:(nb + 1) * 512], ps, b_sb[bname], None, ALU.add
            )

        def emit_v_proj(kb):
            psv = ps2.tile([128, H], F32, tag="pv")
            for ei in range(EC):
                nc.tensor.matmul(
                    psv,
                    lhsT=XT[:, ei, kb * 128:(kb + 1) * 128],
                    rhs=w_sb["Wv"][:, ei, :],
                    start=(ei == 0),
                    stop=False,
                )
            nc.tensor.matmul(
                psv,
                lhsT=ones_row[:, 0:128],
                rhs=b_sb["bv"],
                start=False,
                stop=True,
            )
            nc.vector.tensor_copy(vA[:, kb, 0:H], psv)
            nc.vector.memset(vA[:, kb, H:H + 1], 1.0)

        sTs = {}
        with tc.tile_pool(name="ps2", bufs=2, space="PSUM") as ps2:
            for nb in range(4):
                emit_qk_chunk("Wq", "bq", qT, IT, nb)
            for nb in (0, 1):
                emit_qk_chunk("Wk", "bk", kT, XT, nb)
            # scores for the first x-half run while the second half loads
            for ki in range(8):
                sTs[ki] = emit_score(ki)
            for nb in (2, 3):
                emit_qk_chunk("Wk", "bk", kT, XT, nb)
            for kb in range(SC):
                emit_v_proj(kb)

        # ---- ctx accumulation (PSUM banks freed by ps2 close) ----
        psctx = ctx.enter_context(tc.tile_pool(name="psctx", bufs=1, space="PSUM"))

        # [q_within, qj, 64 ctx + 1 denom + pad] — 128-wide regions keep each
        # accumulation group inside one PSUM bank.
        ctxall = psctx.tile([128, SC, 128], F32, tag="ctxall")

        def emit_ctx(ki):
            # start=True zeroes the whole 2KB PSUM bank, so only the first
            # matmul touching each bank (4 qj regions per bank) gets it; the
            # other ki=0 writes land on zeroed-has_written elements and
            # overwrite. stop on the bank's last matmul.
            sT_sb = sTs.pop(ki)
            for qj in range(SC):
                nc.tensor.matmul(
                    ctxall[:, qj, 0:H + 1],
                    lhsT=sT_sb[:, qj * 128:(qj + 1) * 128],
                    rhs=vA[:, ki, 0:H + 1],
                    start=(ki == 0 and qj % 4 == 0),
                    stop=(ki == SC - 1 and qj % 4 == 3),
                )

        for ki in range(8, SC):
            sTs[ki] = emit_score(ki)
            emit_ctx(ki - 8)
        for ki in range(8, SC):
            emit_ctx(ki)

        # vectorized epilogue: one reciprocal over all 16 denominators, one
        # free-dim-broadcast multiply, one 512KB output DMA
        recip_t = outp.tile([128, SC, 1], F32, tag="recip")
        nc.vector.reciprocal(recip_t, ctxall[:, :, H:H + 1])
        recip_bcast = bass.AP(
            tensor=recip_t.tensor,
            offset=recip_t.offset,
            ap=[recip_t.ap[0], recip_t.ap[1], [0, H]],
        )
        o_all = outp.tile([128, SC, H], F32, tag="o")
        nc.vector.tensor_tensor(o_all, ctxall[:, :, 0:H], recip_bcast, ALU.mult)
        nc.sync.dma_start(
            out=dout.ap().rearrange("(qj p) h -> p qj h", p=128), in_=o_all
        )

    nc.compile()
    return nc


def get_program():
    if "nc" not in _cache:
        _cache["nc"] = _build_program()
    return _cache["nc"]


def make_in_maps(I, x, mask, Wq, bq, Wk, bk, Wv, bv):
    I = np.asarray(I, dtype=np.float32)
    x = np.asarray(x, dtype=np.float32)
    mask = np.asarray(mask, dtype=np.int32)
    import ml_dtypes

    BF = ml_dtypes.bfloat16
    Wpack = np.concatenate(
        [
            np.asarray(Wq, dtype=np.float32).astype(BF),
            np.asarray(Wk, dtype=np.float32).astype(BF),
            np.asarray(Wv, dtype=np.float32).astype(BF),
        ],
        axis=1,
    )
    bpack = np.concatenate(
        [
            np.broadcast_to(np.asarray(bq, np.float32).reshape(H, 1), (H, H)),
            np.broadcast_to(np.asarray(bk, np.float32).reshape(H, 1), (H, H)),
        ],
        axis=1,
    ).astype(np.float32)
    bv = np.asarray(bv, dtype=np.float32).reshape(1, H).astype(BF)

    return [
        {
            "IT": np.ascontiguousarray(I[b].T),
            "XT": np.ascontiguousarray(x[b].T),
            "maskT": np.ascontiguousarray(mask[b].T).astype(np.uint8),
            "Wpack": Wpack, "bpack": bpack, "bv": bv,
        }
        for b in range(B)
    ]


def kernel(I, x, mask, Wq, bq, Wk, bk, Wv, bv):
    nc = get_program()
    in_maps = make_in_maps(I, x, mask, Wq, bq, Wk, bk, Wv, bv)
    res = run_bass_kernel_spmd(nc, in_maps, list(range(N_CORES)))
    out = np.stack([res.results[b]["out"] for b in range(B)], axis=0)
    return out.astype(np.float32)



# revision 4
# speedup vs baseline: 1.1144x; 1.1144x over previous
"""Trainium2 Bass kernel for a single DeBERTa-style attention head (v2).

Problem shapes (hardcoded):
  B=8, S=2048, E=768(n_embed), H=64(head)
  q = I @ Wq + bq ; k = x @ Wk + bk ; v = x @ Wv + bv
  w = (q @ k^T) / sqrt(E) ; w = where(mask==0, -1e9, w)
  scores = softmax(w, axis=-1) ; out = scores @ v

Sharding: data-parallel over batch B across the 8 NeuronCores (one batch
element per core, identical SPMD program).

v2 design notes (vs the v1 baseline at ~100us):
  * Host pre-casts I^T/x^T to bf16 (halves the 12.6MB of fp32 input reads;
    v1 burned ~30us of dead time loading inputs before any compute).
  * bk is dropped entirely (adds a per-q constant to each softmax row -
    shift-invariant); bv is applied on the host after the division; only bq
    is applied on-device.  The device returns the unnormalized context and
    the softmax denominator ([65, S] fp32); the host divides.
  * Input DMAs are issued in 512-seq blocks, IT/XT interleaved, so the
    q/k/v projections and the score pipeline start ~2us in instead of
    waiting for the full load.
  * Score matmuls (contraction dim H=64, half the PE array) run as
    row-tiled pairs: even k-chunks' kT lives on partitions 0-63 and odd
    chunks' on 64-127 (one small SBUF->SBUF DMA per block), with qT
    duplicated onto both halves, so two chunks' score matmuls occupy
    disjoint PE row groups and run concurrently.
  * The ctx matmul is flipped to vA-stationary (16 LDWEIGHTS total instead
    of 256): ctxT[h, q] += vA_c^T @ sT_c streaming the scores.  vA carries
    a ones column so the softmax denominator accumulates as row 64.
  * exp runs on ACT in [128, 1024] windows straight from PSUM (no row max
    needed: |w/sqrt(E)| is O(1)); mask multiply on DVE in bf16 (2x mode).
    ACT is the pacing engine (~35us of exp); PE/DMA/DVE hide under it.
  * Window order is h-major (all q-cols 0-1023 for every chunk pair, then
    q-cols 1024-2047) so the first exp only needs IT blocks 0-1.
  * PSUM: 2x [128,1024] score buffers (4 banks) ping-pong to keep ACT
    busy + ctx accumulator [128(65 used), 2048] (4 banks) = all 8 banks.
"""

import math
from contextlib import ExitStack

import numpy as np

import concourse.bass as bass
import concourse.tile as tile
import concourse.mybir as mybir
from concourse import bacc
from concourse.bass_utils import run_bass_kernel_spmd

B, S, E, H = 8, 2048, 768, 64
N_CORES = 8
SC = S // 128   # 16 seq chunks
EC = E // 128   # 6 embed chunks
NB = 4          # 512-col seq blocks
SCALE = 1.0 / math.sqrt(E)

F32 = mybir.dt.float32
BF16 = mybir.dt.bfloat16
AF = mybir.ActivationFunctionType
ALU = mybir.AluOpType

_cache = {}


def _build_program():
    nc = bacc.Bacc("TRN2", target_bir_lowering=False, debug=False)

    dIT = nc.dram_tensor("IT", [E, S], BF16, kind="ExternalInput")
    dXT = nc.dram_tensor("XT", [E, S], BF16, kind="ExternalInput")
    dmT = nc.dram_tensor("maskT", [S, S], mybir.dt.uint8, kind="ExternalInput")
    dW = nc.dram_tensor("Wpack", [E, 3 * H], BF16, kind="ExternalInput")
    dbq = nc.dram_tensor("bq2", [128, 1], F32, kind="ExternalInput")
    # rows 0-63: unnormalized context^T; row 64: softmax denominator
    dout = nc.dram_tensor("out", [H + 1, S], F32, kind="ExternalOutput")

    with tile.TileContext(nc) as tc, ExitStack() as ctx:
        singles = ctx.enter_context(tc.tile_pool(name="singles", bufs=1))

        IT = singles.tile([128, EC, S], BF16, tag="IT")
        XT = singles.tile([128, EC, S], BF16, tag="XT")
        w_all = singles.tile([128, EC, 3 * H], BF16, tag="Wpack")
        bq2 = singles.tile([128, 1], F32, tag="bq2")
        qT2 = singles.tile([128, S], BF16, tag="qT2")
        kT = singles.tile([64, S], BF16, tag="kT")
        kodd = singles.tile([128, S // 2], BF16, tag="kodd")
        vA = singles.tile([128, SC, 66], BF16, tag="vA")
        out_sb = singles.tile([H + 1, S], F32, tag="out_sb")

        # Trigger the ACT exp table load (~2.7us) while the DMAs stream.
        warm_i = singles.tile([1, 16], F32, tag="warm_i")
        warm_o = singles.tile([1, 16], BF16, tag="warm_o")
        nc.vector.memset(warm_i, 0.0)
        nc.scalar.activation(warm_o, warm_i, AF.Exp)

        nc.vector.memset(vA[:, :, 64:65], 1.0)

        # ---- input DMAs ----
        # Weights first (small, needed by the first projection matmul),
        # then IT/XT interleaved per 512-seq block on the sync HWDGE queue.
        nc.sync.dma_start(
            out=w_all, in_=dW.ap().rearrange("(ec p) h -> p ec h", p=128)
        )
        nc.sync.dma_start(out=bq2, in_=dbq.ap())
        for nb in range(NB):
            sl = slice(nb * 512, (nb + 1) * 512)
            nc.sync.dma_start(
                out=IT[:, :, sl],
                in_=dIT.ap()[:, sl].rearrange("(ec p) s -> p ec s", p=128),
            )
            nc.sync.dma_start(
                out=XT[:, :, sl],
                in_=dXT.ap()[:, sl].rearrange("(ec p) s -> p ec s", p=128),
            )

        # mask chunks: uint8 HBM -> bf16 SBUF cast-DMAs (SWDGE); all 16
        # stay resident (h-major window order uses each chunk twice, ~18us
        # apart).
        mpool = ctx.enter_context(tc.tile_pool(name="mpool", bufs=16))
        m_tiles = []
        for c in range(SC):
            mt = mpool.tile([128, S], BF16, tag="m")
            nc.gpsimd.dma_start(out=mt, in_=dmT.ap()[c * 128:(c + 1) * 128, :])
            m_tiles.append(mt)

        # ---- projections ----
        # q/k/v psum lives in one scoped pool so the score buffers and ctx
        # accumulator can take its banks once projections drain.
        proj_scope = ExitStack()
        psproj = proj_scope.enter_context(
            tc.tile_pool(name="psproj", bufs=2, space="PSUM")
        )

        for nb in range(NB):
            sl = slice(nb * 512, (nb + 1) * 512)
            psq = psproj.tile([64, 512], F32, tag="pq")
            for ei in range(EC):
                nc.tensor.matmul(
                    psq,
                    lhsT=w_all[:, ei, 0:H],
                    rhs=IT[:, ei, sl],
                    start=(ei == 0),
                    stop=(ei == EC - 1),
                )
            # bias fold on the PSUM->SBUF copy; then duplicate onto
            # partitions 64-127 (scalar HWDGE queue - separate FIFO from
            # the big input loads).
            nc.vector.tensor_scalar(qT2[0:64, sl], psq, bq2[0:64], None, ALU.add)
            nc.scalar.dma_start(out=qT2[64:128, sl], in_=qT2[0:64, sl])

        for nb in range(NB):
            sl = slice(nb * 512, (nb + 1) * 512)
            psk = psproj.tile([64, 512], F32, tag="pq")
            for ei in range(EC):
                nc.tensor.matmul(
                    psk,
                    lhsT=w_all[:, ei, H:2 * H],
                    rhs=XT[:, ei, sl],
                    start=(ei == 0),
                    stop=(ei == EC - 1),
                )
            nc.vector.tensor_copy(kT[:, sl], psk)
            # odd chunks of this block -> partitions 64-127 of kodd, packed
            # so pair r sits at cols [r*128, (r+1)*128).
            nc.scalar.dma_start(
                out=kodd[64:128, nb * 256:(nb + 1) * 256].rearrange(
                    "p (two c) -> p two c", c=128
                ),
                in_=kT[:, sl].rearrange("p (four c) -> p four c", c=128)[:, 1::2, :],
            )

        for c in range(SC):
            psv = psproj.tile([128, H], F32, tag="pv")
            for ei in range(EC):
                nc.tensor.matmul(
                    psv,
                    lhsT=XT[:, ei, c * 128:(c + 1) * 128],
                    rhs=w_all[:, ei, 2 * H:3 * H],
                    start=(ei == 0),
                    stop=(ei == EC - 1),
                )
            nc.vector.tensor_copy(vA[:, c, 0:H], psv)

        proj_scope.close()

        # ---- score / softmax / ctx pipeline ----
        wpool = ctx.enter_context(tc.tile_pool(name="wpool", bufs=2, space="PSUM"))
        epool = ctx.enter_context(tc.tile_pool(name="epool", bufs=3))
        spool = ctx.enter_context(tc.tile_pool(name="spool", bufs=16))
        psctx = ctx.enter_context(tc.tile_pool(name="psctx", bufs=1, space="PSUM"))
        ctxall = psctx.tile([128, S], F32, tag="ctxall")

        for h in range(2):
            for r in range(SC // 2):
                for j, c in ((0, 2 * r), (1, 2 * r + 1)):
                    wt = wpool.tile([128, 1024], F32, tag="w")
                    for qb in range(2):
                        col = h * 1024 + qb * 512
                        if j == 0:
                            lhsT = kT[:, c * 128:(c + 1) * 128]
                            rhs = qT2[0:64, col:col + 512]
                        else:
                            lhsT = kodd[64:128, r * 128:(r + 1) * 128]
                            rhs = qT2[64:128, col:col + 512]
                        nc.tensor.matmul(
                            wt[:, qb * 512:(qb + 1) * 512],
                            lhsT=lhsT,
                            rhs=rhs,
                            start=True,
                            stop=True,
                        )
                    et = epool.tile([128, 1024], BF16, tag="e")
                    nc.scalar.activation(et, wt, AF.Exp, scale=SCALE)
                    st = spool.tile([128, 1024], BF16, tag="s")
                    nc.vector.tensor_tensor(
                        st, et, m_tiles[c][:, h * 1024:(h + 1) * 1024], ALU.mult
                    )
                    for qb in range(2):
                        col = h * 1024 + qb * 512
                        nc.tensor.matmul(
                            ctxall[0:H + 1, col:col + 512],
                            lhsT=vA[:, c, 0:H + 1],
                            rhs=st[:, qb * 512:(qb + 1) * 512],
                            start=(r == 0 and j == 0),
                            stop=(r == SC // 2 - 1 and j == 1),
                        )
            # this q-half's two ctx banks are final: drain and ship
            hs = slice(h * 1024, (h + 1) * 1024)
            nc.vector.tensor_copy(out_sb[:, hs], ctxall[0:H + 1, hs])
            nc.sync.dma_start(out=dout.ap()[:, hs], in_=out_sb[:, hs])

    nc.compile()
    return nc


def get_program():
    if "nc" not in _cache:
        _cache["nc"] = _build_program()
    return _cache["nc"]


def make_in_maps(I, x, mask, Wq, bq, Wk, bk, Wv, bv):
    import ml_dtypes

    BF = ml_dtypes.bfloat16
    I = np.asarray(I, dtype=np.float32)
    x = np.asarray(x, dtype=np.float32)
    mask = np.asarray(mask, dtype=np.int32)
    Wpack = np.concatenate(
        [
            np.asarray(Wq, dtype=np.float32),
            np.asarray(Wk, dtype=np.float32),
            np.asarray(Wv, dtype=np.float32),
        ],
        axis=1,
    ).astype(BF)
    bq2 = np.tile(np.asarray(bq, np.float32).reshape(H, 1), (2, 1))

    return [
        {
            "IT": np.ascontiguousarray(I[b].T).astype(BF),
            "XT": np.ascontiguousarray(x[b].T).astype(BF),
            "maskT": np.ascontiguousarray(mask[b].T).astype(np.uint8),
            "Wpack": Wpack,
            "bq2": bq2,
        }
        for b in range(B)
    ]


def postprocess(raw, bv):
    """raw: [65, S] f32 (64 ctx rows + denominator). Returns [S, H] f32."""
    return (raw[0:H] / raw[H:H + 1]).T + np.asarray(bv, np.float32)


def kernel(I, x, mask, Wq, bq, Wk, bk, Wv, bv):
    nc = get_program()
    in_maps = make_in_maps(I, x, mask, Wq, bq, Wk, bk, Wv, bv)
    res = run_bass_kernel_spmd(nc, in_maps, list(range(N_CORES)))
    out = np.stack(
        [postprocess(res.results[b]["out"], bv) for b in range(B)], axis=0
    )
    return out.astype(np.float32)


# revision 8
# speedup vs baseline: 1.1348x; 1.0183x over previous
"""Trainium2 Bass kernel for a single DeBERTa-style attention head (v3).

Problem shapes (hardcoded):
  B=8, S=2048, E=768(n_embed), H=64(head)
  q = I @ Wq + bq ; k = x @ Wk + bk ; v = x @ Wv + bv
  w = (q @ k^T) / sqrt(E) ; w = where(mask==0, -1e9, w)
  scores = softmax(w, axis=-1) ; out = scores @ v

Sharding: data-parallel over batch B across the 8 NeuronCores (one batch
element per core, identical SPMD program).

The kernel is paced by the exp chain on ACT (32 x [128,1024] windows
~= 35us); everything else is arranged to hide under it:
  * Host pre-packs all inputs bf16 **partition-major** so every DMA moves
    multi-KB contiguous runs per partition (few fat descriptors; the 1KB
    descriptors of the naive layout capped DMA at ~250 GB/s).
  * I^T/x^T arrive as two seq-halves each, interleaved (ITa, XTa, XTb,
    ITb) on the sync HWDGE queue; mask chunk-group cast-DMAs (SWDGE) are
    gated behind input arrival by tiny gpsimd reduces writing the first
    element of each mask tile, so they cannot steal early bandwidth.
  * bk is dropped (softmax shift-invariant), bv applied on host, bq folded
    into the q PSUM->SBUF copy.  The device returns unnormalized context^T
    plus the denominator row ([65, S] fp32); the host divides.
  * Scores (K=64) run as row-tiled pairs: even chunks' kT on partitions
    0-63, odd on 64-127 (small SBUF->SBUF DMAs on the scalar HWDGE ring),
    qT duplicated onto both halves -> two chunks per PE pass.
  * ctx is vA-stationary (16 LDWEIGHTS total); vA carries a ones column so
    row 64 of the accumulator is the softmax denominator.
  * Window order is h-major (q-cols 0-1023 for all 16 chunks, then
    1024-2047) so the pipeline starts after only ITa+XTa; the q half-b
    projection runs mid-pipeline borrowing a score PSUM buffer.
  * PSUM: score pool 2x[128,1024] (4 banks) + projections (4 banks, later
    reused by the ctx accumulator) = 8 banks.
"""

import math
from contextlib import ExitStack

import numpy as np

import concourse.bass as bass
import concourse.tile as tile
import concourse.mybir as mybir
from concourse import bacc
from concourse.bass_utils import run_bass_kernel_spmd

B, S, E, H = 8, 2048, 768, 64
N_CORES = 8
SC = S // 128   # 16 seq chunks
EC = E // 128   # 6 embed chunks
SCALE = 1.0 / math.sqrt(E)

F32 = mybir.dt.float32
BF16 = mybir.dt.bfloat16
AF = mybir.ActivationFunctionType
ALU = mybir.AluOpType
AX = mybir.AxisListType

_cache = {}


def _build_program():
    nc = bacc.Bacc("TRN2", target_bir_lowering=False, debug=False)

    # seq-halves, partition-major: [half, p, ec, 1024]
    dIT = nc.dram_tensor("IT", [2, 128, EC, 1024], BF16, kind="ExternalInput")
    dXT = nc.dram_tensor("XT", [2, 128, EC, 1024], BF16, kind="ExternalInput")
    # [p, chunk, q] so a 4-chunk group is an 8KB contiguous run per partition
    dmT = nc.dram_tensor("maskT", [128, SC, S], mybir.dt.uint8, kind="ExternalInput")
    dW = nc.dram_tensor("Wpack", [E, 3 * H], BF16, kind="ExternalInput")
    dbq = nc.dram_tensor("bq2", [128, 1], F32, kind="ExternalInput")
    # rows 0-63: unnormalized context^T; row 64: softmax denominator
    dout = nc.dram_tensor("out", [H + 1, S], F32, kind="ExternalOutput")

    with tile.TileContext(nc) as tc, ExitStack() as ctx:
        singles = ctx.enter_context(tc.tile_pool(name="singles", bufs=1))

        # [p, half, ec, 1024] so each DMA half is one 12KB run per partition
        IT = singles.tile([128, 2, EC, 1024], BF16, tag="IT")
        XT = singles.tile([128, 2, EC, 1024], BF16, tag="XT")
        w_all = singles.tile([128, EC, 3 * H], BF16, tag="Wpack")
        bq2 = singles.tile([128, 1], F32, tag="bq2")
        qT2 = singles.tile([128, S], BF16, tag="qT2")
        kT = singles.tile([64, S], BF16, tag="kT")
        kodd = singles.tile([128, S // 2], BF16, tag="kodd")
        vA = singles.tile([128, SC, 66], BF16, tag="vA")
        out_sb = singles.tile([H + 1, S], F32, tag="out_sb")

        # Trigger the ACT exp table load (~2.7us) while the DMAs stream.
        warm_i = singles.tile([1, 16], F32, tag="warm_i")
        warm_o = singles.tile([1, 16], BF16, tag="warm_o")
        nc.vector.memset(warm_i, 0.0)
        nc.scalar.activation(warm_o, warm_i, AF.Exp)

        nc.vector.memset(vA[:, :, 64:65], 1.0)

        # ---- input DMAs (sync HWDGE queue, FIFO) ----
        nc.sync.dma_start(
            out=w_all, in_=dW.ap().rearrange("(ec p) h -> p ec h", p=128)
        )
        nc.sync.dma_start(out=bq2, in_=dbq.ap())
        nc.sync.dma_start(out=IT[:, 0], in_=dIT.ap()[0])
        nc.sync.dma_start(out=XT[:, 0], in_=dXT.ap()[0])
        nc.sync.dma_start(out=XT[:, 1], in_=dXT.ap()[1])
        nc.sync.dma_start(out=IT[:, 1], in_=dIT.ap()[1])

        # ---- mask cast-DMAs in 4-chunk groups, gated so they can't
        # starve the input stream: a tiny gpsimd partition-reduce writes
        # the tile's first element after the gate input lands (WAW dep for
        # the DMA + Pool-FIFO ordering of the triggers).
        mpool = ctx.enter_context(tc.tile_pool(name="mpool", bufs=4))
        gates = [IT[0:1, 0, 0, 0:1], IT[0:1, 0, 0, 1:2], XT[0:1, 0, 0, 0:1],
                 XT[0:1, 1, 0, 0:1]]
        m_groups = []
        for g in range(4):
            mt = mpool.tile([128, 4, S], BF16, tag="m")
            nc.gpsimd.tensor_reduce(mt[0:1, 0:1, 0:1], gates[g], AX.C, ALU.max)
            nc.gpsimd.dma_start(out=mt, in_=dmT.ap()[:, g * 4:(g + 1) * 4, :])
            m_groups.append(mt)

        def m_tile(c):
            return m_groups[c // 4][:, c % 4, :]

        # ---- PSUM pools: projections [0,8K) + score windows [8K,16K);
        # the ctx accumulator later reuses the projection banks.
        wpool = ctx.enter_context(tc.tile_pool(name="wpool", bufs=2, space="PSUM"))
        proj_scope = ExitStack()
        psproj = proj_scope.enter_context(
            tc.tile_pool(name="psproj", bufs=2, space="PSUM")
        )

        # ---- projections (emission order = engine-queue order: q half-a
        # first since ITa lands first, then the XTa-, then XTb-dependent
        # work) ----
        def emit_q(half, mk_psq):
            for blk in range(2):
                psq = mk_psq()
                for ei in range(EC):
                    nc.tensor.matmul(
                        psq,
                        lhsT=w_all[:, ei, 0:H],
                        rhs=IT[:, half, ei, blk * 512:(blk + 1) * 512],
                        start=(ei == 0),
                        stop=(ei == EC - 1),
                    )
                sl = slice(half * 1024 + blk * 512, half * 1024 + (blk + 1) * 512)
                nc.vector.tensor_scalar(qT2[0:64, sl], psq, bq2[0:64], None, ALU.add)
            sl = slice(half * 1024, (half + 1) * 1024)
            nc.scalar.dma_start(out=qT2[64:128, sl], in_=qT2[0:64, sl])

        emit_q(0, lambda: psproj.tile([64, 512], F32, tag="pk", name="psq"))

        def emit_k(half):
            for blk in range(2):
                nbsl = slice(half * 1024 + blk * 512, half * 1024 + (blk + 1) * 512)
                psk = psproj.tile([64, 512], F32, tag="pk")
                for ei in range(EC):
                    nc.tensor.matmul(
                        psk,
                        lhsT=w_all[:, ei, H:2 * H],
                        rhs=XT[:, half, ei, blk * 512:(blk + 1) * 512],
                        start=(ei == 0),
                        stop=(ei == EC - 1),
                    )
                nc.vector.tensor_copy(kT[:, nbsl], psk)
            # odd chunks of this half -> partitions 64-127 of kodd, packed
            # so pair r sits at cols [r*128, (r+1)*128)
            nc.scalar.dma_start(
                out=kodd[64:128, half * 512:(half + 1) * 512].rearrange(
                    "p (four c) -> p four c", c=128
                ),
                in_=kT[:, half * 1024:(half + 1) * 1024].rearrange(
                    "p (eight c) -> p eight c", c=128
                )[:, 1::2, :],
            )

        def emit_v(half):
            for lc in range(8):
                c = half * 8 + lc
                psv = psproj.tile([128, H], F32, tag="pv")
                for ei in range(EC):
                    nc.tensor.matmul(
                        psv,
                        lhsT=XT[:, half, ei, lc * 128:(lc + 1) * 128],
                        rhs=w_all[:, ei, 2 * H:3 * H],
                        start=(ei == 0),
                        stop=(ei == EC - 1),
                    )
                nc.vector.tensor_copy(vA[:, c, 0:H], psv)

        emit_k(0)
        emit_v(0)
        emit_k(1)
        emit_v(1)

        proj_scope.close()

        # ---- score / softmax / ctx pipeline ----
        epool = ctx.enter_context(tc.tile_pool(name="epool", bufs=3))
        spool = ctx.enter_context(tc.tile_pool(name="spool", bufs=16))
        psctx = ctx.enter_context(tc.tile_pool(name="psctx", bufs=1, space="PSUM"))
        ctxall = psctx.tile([128, S], F32, tag="ctxall")

        def emit_window(h, r, j, c):
            wt = wpool.tile([128, 1024], F32, tag="w")
            for qb in range(2):
                col = h * 1024 + qb * 512
                if j == 0:
                    lhsT = kT[:, c * 128:(c + 1) * 128]
                    rhs = qT2[0:64, col:col + 512]
                else:
                    lhsT = kodd[64:128, r * 128:(r + 1) * 128]
                    rhs = qT2[64:128, col:col + 512]
                nc.tensor.matmul(
                    wt[:, qb * 512:(qb + 1) * 512],
                    lhsT=lhsT, rhs=rhs, start=True, stop=True,
                )
            et = epool.tile([128, 1024], BF16, tag="e")
            nc.scalar.activation(et, wt, AF.Exp, scale=SCALE)
            st = spool.tile([128, 1024], BF16, tag="s")
            nc.vector.tensor_tensor(
                st, et, m_tile(c)[:, h * 1024:(h + 1) * 1024], ALU.mult
            )
            for qb in range(2):
                col = h * 1024 + qb * 512
                nc.tensor.matmul(
                    ctxall[0:H + 1, col:col + 512],
                    lhsT=vA[:, c, 0:H + 1],
                    rhs=st[:, qb * 512:(qb + 1) * 512],
                    start=(r == 0 and j == 0),
                    stop=(r == SC // 2 - 1 and j == 1),
                )

        def emit_drain(h):
            hs = slice(h * 1024, (h + 1) * 1024)
            nc.vector.tensor_copy(out_sb[:, hs], ctxall[0:H + 1, hs])
            nc.sync.dma_start(out=dout.ap()[:, hs], in_=out_sb[:, hs])

        for r in range(SC // 2):
            for j in range(2):
                emit_window(0, r, j, 2 * r + j)
            if r == 5:
                # q half-b projection, borrowing score PSUM buffers;
                # lands well before window 16 needs it.
                emit_q(1, lambda: wpool.tile([128, 1024], F32, tag="w",
                                             name="psq_b")[0:64, 0:512])
        emit_drain(0)
        for r in range(SC // 2):
            for j in range(2):
                emit_window(1, r, j, 2 * r + j)
        emit_drain(1)

    nc.compile()
    return nc


def get_program():
    if "nc" not in _cache:
        _cache["nc"] = _build_program()
    return _cache["nc"]


def make_in_maps(I, x, mask, Wq, bq, Wk, bk, Wv, bv):
    import ml_dtypes

    BF = ml_dtypes.bfloat16
    I = np.asarray(I, dtype=np.float32)
    x = np.asarray(x, dtype=np.float32)
    mask = np.asarray(mask, dtype=np.int32)
    Wpack = np.concatenate(
        [
            np.asarray(Wq, dtype=np.float32),
            np.asarray(Wk, dtype=np.float32),
            np.asarray(Wv, dtype=np.float32),
        ],
        axis=1,
    ).astype(BF)
    bq2 = np.tile(np.asarray(bq, np.float32).reshape(H, 1), (2, 1))

    def pack_ixt(a):
        # [S, E] -> [half, p, ec, 1024] partition-major halves of a^T
        t = np.ascontiguousarray(a.T).astype(BF).reshape(EC, 128, S)
        return np.ascontiguousarray(
            np.stack([t[:, :, 0:1024], t[:, :, 1024:2048]], 0).transpose(0, 2, 1, 3)
        )

    def pack_mask(m):
        # [S(q), S(k)] -> [p, chunk, q] of mask^T
        return np.ascontiguousarray(
            m.T.astype(np.uint8).reshape(SC, 128, S).transpose(1, 0, 2)
        )

    return [
        {
            "IT": pack_ixt(I[b]),
            "XT": pack_ixt(x[b]),
            "maskT": pack_mask(mask[b]),
            "Wpack": Wpack,
            "bq2": bq2,
        }
        for b in range(B)
    ]


def postprocess(raw, bv):
    """raw: [65, S] f32 (64 ctx rows + denominator). Returns [S, H] f32."""
    return (raw[0:H] / raw[H:H + 1]).T + np.asarray(bv, np.float32)


def kernel(I, x, mask, Wq, bq, Wk, bk, Wv, bv):
    nc = get_program()
    in_maps = make_in_maps(I, x, mask, Wq, bq, Wk, bk, Wv, bv)
    res = run_bass_kernel_spmd(nc, in_maps, list(range(N_CORES)))
    out = np.stack(
        [postprocess(res.results[b]["out"], bv) for b in range(B)], axis=0
    )
    return out.astype(np.float32)
